# revision 2
# baseline (speedup 1.0000x reference)
"""Trainium2 Bass kernel v2 for nn_MultiHeadAttention (B=2, T=4096, E=512, H=8, dk=dv=64).

Sharding: 8 cores = 2 batches x 4 query-row chunks (same as baseline).

Design (per core):
  - Inputs bf16; X^T tiles via HWDGE DMA-transpose on the SP/ACT queues.
  - Projections bf16 (eb-chained N=512 MMs into PSUM), bias folded on the
    PSUM->SBUF copy (DVE/ACT alternating). K proj jb0 + V chunks 0..7 run
    before attention; the rest is interleaved into the attention loop to
    fill PE idle time (the attention loop is exp-bound).
  - Scores: S^T[s,t] = kT_h.T @ qT_h per (head, s-chunk), K=64, with the
    head pair row-tiled at partition bases 0/64 (concurrent on PE).
  - exp: split across ACT (true exp -> fp8e4 out) and DVE (Schraudolph:
    one fused tensor_scalar affine -> int8 bits == fp8e4 of exp). Both
    write slots of a [128, 2, 1024] fp8 E tile (s-chunk pairs).
  - PV: fp8e4 DoubleRow matmuls, contraction 256 s-rows per MM. The
    stationary operand is vS = [v_h | ones | pad] (128 cols per k-slot,
    col 64 = ones) so PSUM row 64 accumulates the softmax denominator.
  - Normalization per pair: ACT gathers den rows from PSUM, DVE
    reciprocal_approx_fast, gpsimd partition_broadcast, DVE multiplies
    PSUM numerators by the broadcast reciprocals into aT (bf16).
  - Output projection at the tail: lhsT = aT chunks, + bf16 ones-row bias MM.
"""

import os
import sys
from contextlib import ExitStack

for _p in ("/opt/trn_rl_repo",):
    if _p not in sys.path and os.path.isdir(_p):
        sys.path.insert(0, _p)

import numpy as np

import concourse.bass as bass
import concourse.mybir as mybir
import concourse.tile as tile
from concourse import bacc
from concourse.bass_utils import run_bass_kernel_spmd

F32 = mybir.dt.float32
BF16 = mybir.dt.bfloat16
FP8 = mybir.dt.float8e4
I8 = mybir.dt.int8
AF = mybir.ActivationFunctionType
ALU = mybir.AluOpType
DR = mybir.MatmulPerfMode.DoubleRow

B, T, E, H, D = 2, 4096, 512, 8, 64
TL = T // 4          # query rows per core
S = T                # kv rows per core
P = 128
NSC = S // P         # 32 s-chunks
NSP = NSC // 2       # 16 s-chunk pairs
NCORES = 8

LN2 = float(np.log(2.0))
SCH_C = float(os.environ.get("MHA_SCH_C", "0.5"))
STAGE = "full"
A8 = 1.0 / LN2                 # scores * 0.125 * 8 / ln2
B8 = 7.0 * 8.0 - SCH_C


def build_nc():
    nc = bacc.Bacc(
        "TRN2",
        target_bir_lowering=False,
        debug=False,
        enable_asserts=False,
        num_devices=NCORES,
    )

    q_d = nc.dram_tensor("q", [TL, E], BF16, kind="ExternalInput").ap()
    k_d = nc.dram_tensor("k", [S, E], BF16, kind="ExternalInput").ap()
    v_d = nc.dram_tensor("v", [S, E], BF16, kind="ExternalInput").ap()
    w_d = {
        n: nc.dram_tensor(n, [E, E], BF16, kind="ExternalInput").ap()
        for n in ("wq", "wk", "wv", "wo")
    }
    bq_d = nc.dram_tensor("bq", [1, E], F32, kind="ExternalInput").ap()
    bk_d = nc.dram_tensor("bk", [1, E], F32, kind="ExternalInput").ap()
    bo_d = nc.dram_tensor("bo", [1, E], BF16, kind="ExternalInput").ap()
    out_d = nc.dram_tensor("out", [TL, E], F32, kind="ExternalOutput").ap()
    dbg = {}
    if STAGE == "debug":
        dbg["qT"] = nc.dram_tensor("dbg_qT", [P, 4 * TL], BF16, kind="ExternalOutput").ap()
        dbg["kT"] = nc.dram_tensor("dbg_kT", [P, 4 * S], BF16, kind="ExternalOutput").ap()
        dbg["vS"] = nc.dram_tensor("dbg_vS", [P, NSP * 2 * 688], mybir.dt.uint8, kind="ExternalOutput").ap()
        dbg["E0"] = nc.dram_tensor("dbg_E0", [P, 2 * TL], mybir.dt.uint8, kind="ExternalOutput").ap()
        dbg["pv0"] = nc.dram_tensor("dbg_pv0", [P, 2048], F32, kind="ExternalOutput").ap()
        dbg["dt"] = nc.dram_tensor("dbg_dt", [1, 2048], F32, kind="ExternalOutput").ap()
        dbg["rdp"] = nc.dram_tensor("dbg_rdp", [1, 2048], F32, kind="ExternalOutput").ap()
        dbg["rdb"] = nc.dram_tensor("dbg_rdb", [64, 2048], F32, kind="ExternalOutput").ap()
        dbg["aT"] = nc.dram_tensor("dbg_aT", [P, 4 * TL], BF16, kind="ExternalOutput").ap()
        dbg["xk00"] = nc.dram_tensor("dbg_xk00", [P, S // 2], BF16, kind="ExternalOutput").ap()
        dbg["xk01"] = nc.dram_tensor("dbg_xk01", [P, S // 2], BF16, kind="ExternalOutput").ap()

    with tile.TileContext(nc) as tc, ExitStack() as ctx:
        sb = ctx.enter_context(tc.tile_pool(name="sb", bufs=1))
        ep = ctx.enter_context(tc.tile_pool(name="ep", bufs=6))
        rdbp = ctx.enter_context(tc.tile_pool(name="rdbp", bufs=1))
        rp = ctx.enter_context(tc.tile_pool(name="rp", bufs=1))
        outp = ctx.enter_context(tc.tile_pool(name="outp", bufs=2))
        psS = ctx.enter_context(tc.tile_pool(name="psS", bufs=2, space="PSUM"))
        psV = ctx.enter_context(tc.tile_pool(name="psV", bufs=1, space="PSUM"))

        # ---- static tiles ----
        kT = sb.tile([P, 4, S], BF16, name="kT", tag="kT")
        qT = sb.tile([P, 4, TL], BF16, name="qT", tag="qT")
        VW = 8 * 80 + 48  # 688: 80 cols per head (64 v + 1 ones + 15 pad), +48 tail
        vS = sb.tile([P, NSP, 2, VW], FP8, name="vS", tag="vS")
        aT = sb.tile([P, 4, TL], BF16, name="aT", tag="aT")
        ones_b = sb.tile([1, P], BF16, name="ones_b", tag="ones_b")
        nc.gpsimd.memset(ones_b[:], 1.0)
        # ones column of vS (col 64 of every [.., 128] block)
        nc.gpsimd.memset(vS[:, :, :, 0 : 8 * 80].rearrange("p sp k (h c) -> p sp k h c", c=80)[:, :, :, :, D : D + 1], 1.0)

        bias_c = {}
        for n, d in (("bq", bq_d), ("bk", bk_d)):
            bias_c[n] = sb.tile([P, 4], F32, name=n + "c", tag=n + "c")
            nc.sync.dma_start(
                out=bias_c[n][:], in_=d.rearrange("o (jb p) -> p (jb o)", p=P)
            )
        bo_t = sb.tile([1, E], BF16, name="bo", tag="bo")
        nc.sync.dma_start(out=bo_t[:], in_=bo_d[:])

        # ---- input transposes (HWDGE transpose DMAs: SP/ACT queues only) ----
        tq_flip = [0]

        def tq_eng():
            tq_flip[0] ^= 1
            if os.environ.get("MHA_TQ", "sync") == "sync":
                return nc.sync
            return nc.sync if tq_flip[0] else nc.scalar

        xq_t = []
        for eb in range(4):
            t = sb.tile([P, TL], BF16, name="xq", tag=f"xq{eb}")
            tq_eng().dma_start(
                out=t[:], in_=q_d[:, eb * P : (eb + 1) * P], transpose=True
            )
            xq_t.append(t)
        xk_t = [[None] * 2 for _ in range(4)]
        xv_t = [[None] * 2 for _ in range(4)]
        for hf in range(2):
            for eb in range(4):
                tk = sb.tile([P, S // 2], BF16, name="xk", tag=f"xk{eb}_{hf}")
                tq_eng().dma_start(
                    out=tk[:],
                    in_=k_d[hf * (S // 2) : (hf + 1) * (S // 2), eb * P : (eb + 1) * P],
                    transpose=True,
                )
                xk_t[eb][hf] = tk
            for eb in range(4):
                tv = sb.tile([P, S // 2], BF16, name="xv", tag=f"xv{eb}_{hf}")
                tq_eng().dma_start(
                    out=tv[:],
                    in_=v_d[hf * (S // 2) : (hf + 1) * (S // 2), eb * P : (eb + 1) * P],
                    transpose=True,
                )
                xv_t[eb][hf] = tv

        # ---- weights ----
        w_t = {}
        for n in ("wq", "wk", "wv", "wo"):
            w_t[n] = sb.tile([P, 4, E], BF16, name=n, tag=n)
            nc.gpsimd.dma_start(
                out=w_t[n][:], in_=w_d[n].rearrange("(eb p) j -> p eb j", p=P)
            )

        copy_flip = [0]

        def copy_eng():
            copy_flip[0] ^= 1
            return nc.vector if copy_flip[0] else nc.scalar

        def proj_copy_bias(dst, src, bias_col):
            """psum -> sbuf with per-partition bias add (alternating engine)."""
            if copy_eng() is nc.vector:
                nc.vector.tensor_scalar(dst, src, bias_col, None, ALU.add)
            else:
                nc.scalar.activation(dst, src, AF.Identity, bias=bias_col)

        # ---------- projection groups ----------
        def q_group(jb, tc2):
            pt = psS.tile([P, 1024], F32, name="ps", tag="ps")
            for eb in range(4):
                nc.tensor.matmul(
                    pt[:, 0:512],
                    w_t["wq"][:, eb, jb * P : (jb + 1) * P],
                    xq_t[eb][:, tc2 * 512 : (tc2 + 1) * 512],
                    start=(eb == 0),
                    stop=(eb == 3),
                )
            proj_copy_bias(
                qT[:, jb, tc2 * 512 : (tc2 + 1) * 512], pt[:, 0:512],
                bias_c["bq"][:, jb : jb + 1],
            )

        def k_group(jb, g):
            """K proj for s rows [g*1024, (g+1)*1024)."""
            hf, part = divmod(g, 2)
            pt = psS.tile([P, 1024], F32, name="ps", tag="ps")
            for half in range(2):
                sl = slice(part * 1024 + half * 512, part * 1024 + (half + 1) * 512)
                for eb in range(4):
                    nc.tensor.matmul(
                        pt[:, half * 512 : (half + 1) * 512],
                        w_t["wk"][:, eb, jb * P : (jb + 1) * P],
                        xk_t[eb][hf][:, sl],
                        start=(eb == 0),
                        stop=(eb == 3),
                    )
            proj_copy_bias(
                kT[:, jb, g * 1024 : (g + 1) * 1024], pt[:],
                bias_c["bk"][:, jb : jb + 1],
            )

        def v_group(sp):
            """V proj for s-chunk pair sp (rows sp*256 .. sp*256+255)."""
            hf = sp // 8
            base = (sp % 8) * 256
            pt = psS.tile([P, 1024], F32, name="ps", tag="ps")
            for k01 in range(2):
                sl = slice(base + k01 * P, base + (k01 + 1) * P)
                for eb in range(4):
                    nc.tensor.matmul(
                        pt[:, k01 * 512 : (k01 + 1) * 512],
                        xv_t[eb][hf][:, sl],
                        w_t["wv"][:, eb, :],
                        start=(eb == 0),
                        stop=(eb == 3),
                    )
            # strided copy into vS (cast f32 -> fp8), no bias (bv folded into bo)
            src = pt[:].rearrange("p (k h d) -> p k h d", k=2, d=D)
            dst = vS[:, sp, :, 0 : 8 * 80].rearrange("p k (h c) -> p k h c", c=80)[:, :, :, 0:D]
            if copy_eng() is nc.vector:
                nc.vector.tensor_copy(dst, src)
            else:
                nc.scalar.activation(dst, src, AF.Identity)

        # ---------- emission ----------
        for jb in range(4):
            for tc2 in range(2):
                q_group(jb, tc2)
        for g in range(2):
            k_group(0, g)
        for sp in range(4):
            v_group(sp)

        # Interleaved projection jobs, scheduled so each group is emitted
        # strictly before its first reader:
        #  pair0 pops: v_group(8..15) at sp 0..7 (PV(sp_v) reads at sp_v >= sp_v-8),
        #              then K jb1 (read by pair1)
        #  pair1 pops: K jb2 (read by pair2);  pair2 pops: K jb3
        # One global queue, one job per sp unit. Order satisfies
        # "emitted before first reader" for every group (see per-job comments).
        proj_jobs = (
            [(k_group, (0, 2)), (k_group, (0, 3))]            # read at pair0 sp>=8
            + [(v_group, (sp,)) for sp in range(4, NSP)]      # vS[sp] read at pair0 sp
            + [(k_group, (1, g)) for g in range(4)]           # pops 14-17 < pair1 reads
            + [(k_group, (2, g)) for g in range(4)]           # pops 18-21, pair1 sps 2-5
            + [(k_group, (3, g)) for g in range(4)]           # pops 22-25, pair1 sps 6-9
        )

        def pv_mms(pair, pvt, sp, e_tiles):
            for hl in range(2):
                h = 2 * pair + hl
                et = e_tiles[hl]
                for tc2 in range(2):
                    nc.tensor.matmul(
                        pvt[:, (2 * hl + tc2) * 512 : (2 * hl + tc2 + 1) * 512],
                        vS[:, sp, :, h * 80 : h * 80 + P],
                        et[:, :, tc2 * 512 : (tc2 + 1) * 512],
                        start=(sp == 0),
                        stop=(sp == NSP - 1),
                        perf_mode=DR,
                    )

        for pair in range(4 if STAGE != "proj" else 0):
            jb = pair
            pvt = psV.tile([P, 2048], F32, name="pv", tag="pv")
            prev_e = None
            for sp in range(NSP):
                e_tiles = {}
                for hl in range(2):
                    e_tiles[hl] = ep.tile([P, 2, TL], FP8, name="E", tag="E")
                for k01 in range(2):
                    sc = 2 * sp + k01
                    sts = {}
                    for hl in range(2):
                        sts[hl] = psS.tile([P, TL], F32, name="ps", tag="ps")
                    # interleave the head pair's score MMs so adjacent PE
                    # instructions target disjoint row groups (rows 0-63 vs
                    # 64-127) and overlap in the array.
                    for tc2 in range(2):
                        for hl in range(2):
                            r0 = 64 * hl
                            nc.tensor.matmul(
                                sts[hl][:, tc2 * 512 : (tc2 + 1) * 512],
                                kT[r0 : r0 + 64, jb, sc * P : (sc + 1) * P],
                                qT[r0 : r0 + 64, jb, tc2 * 512 : (tc2 + 1) * 512],
                                start=True,
                                stop=True,
                            )
                    for hl in range(2):
                        h = 2 * pair + hl
                        et = e_tiles[hl]
                        if h & 1:
                            nc.vector.tensor_scalar(
                                et[:, k01, :].bitcast(I8), sts[hl][:],
                                A8, B8, ALU.mult, ALU.add,
                            )
                        else:
                            nc.scalar.activation(et[:, k01, :], sts[hl][:], AF.Exp, scale=0.125)
                if prev_e is not None:
                    pv_mms(pair, pvt, sp - 1, prev_e)
                prev_e = e_tiles
                if proj_jobs:
                    fn, args = proj_jobs.pop(0)
                    fn(*args)
            pv_mms(pair, pvt, NSP - 1, prev_e)

            if STAGE == "debug" and pair == 0:
                pv_sb = rdbp.tile([P, 2048], F32, name="pvdump", tag="pvdump")
                nc.vector.tensor_copy(pv_sb[:], pvt[:])
                nc.sync.dma_start(out=dbg["pv0"][:], in_=pv_sb[:])
            if STAGE == "attn":
                continue
            # ---- pair normalization ----
            # gather the 4 den rows into one [1, 2048] row at partition 0
            dt_t = rp.tile([1, 2048], F32, name="dt", tag="dt")
            nc.scalar.activation(dt_t[:], pvt[64:65, :], AF.Identity)
            rdp = rp.tile([1, 2048], F32, name="rdp", tag="rdp")
            nc.vector.reciprocal_approx_fast(rdp[:], dt_t[:])
            rdb = rdbp.tile([64, 2048], F32, name="rdb", tag="rdb")
            if STAGE == "norm_nobc":
                nc.gpsimd.memset(rdb[:], 0.001)
            else:
                nc.gpsimd.partition_broadcast(rdb[:], rdp[:])
            if STAGE == "debug" and pair == 0:
                nc.sync.dma_start(out=dbg["dt"][:], in_=dt_t[:])
                nc.sync.dma_start(out=dbg["rdp"][:], in_=rdp[:])
                nc.sync.dma_start(out=dbg["rdb"][:], in_=rdb[:])
            for hl in range(2):
                r0 = 64 * hl
                nc.vector.tensor_mul(
                    aT[r0 : r0 + 64, jb, :],
                    pvt[0:64, hl * 1024 : (hl + 1) * 1024],
                    rdb[0:64, hl * 1024 : (hl + 1) * 1024],
                )

        if STAGE == "debug":
            nc.sync.dma_start(out=dbg["qT"][:], in_=qT[:].rearrange("p a t -> p (a t)"))
            nc.sync.dma_start(out=dbg["kT"][:], in_=kT[:].rearrange("p a t -> p (a t)"))
            nc.sync.dma_start(out=dbg["vS"][:], in_=vS[:].rearrange("p a k c -> p (a k c)").bitcast(mybir.dt.uint8))
            nc.sync.dma_start(out=dbg["aT"][:], in_=aT[:].rearrange("p a t -> p (a t)"))
            nc.sync.dma_start(out=dbg["xk00"][:], in_=xk_t[0][0][:])
            nc.sync.dma_start(out=dbg["xk01"][:], in_=xk_t[0][1][:])

        # ---- output projection ----
        for mt in range(TL // P if STAGE in ("full", "norm_nobc", "debug") else 0):
            pt = psS.tile([P, 1024], F32, name="ps", tag="ps")
            for jb in range(4):
                nc.tensor.matmul(
                    pt[:, 0:512],
                    aT[:, jb, mt * P : (mt + 1) * P],
                    w_t["wo"][:, jb, :],
                    start=(jb == 0),
                    stop=False,
                )
            nc.tensor.matmul(
                pt[:, 0:512], ones_b[0:1, 0:P], bo_t[:], start=False, stop=True
            )
            ot = outp.tile([P, 512], F32, name="out", tag="out")
            if copy_eng() is nc.vector:
                nc.vector.tensor_copy(ot[:], pt[:, 0:512])
            else:
                nc.scalar.activation(ot[:], pt[:, 0:512], AF.Identity)
            nc.sync.dma_start(out_d[mt * P : (mt + 1) * P, :], ot[:])

        if STAGE != "full":
            zt = outp.tile([P, 512], F32, name="z", tag="out")
            nc.gpsimd.memset(zt[:], 0.0)
            for mt in range(TL // P):
                nc.sync.dma_start(out_d[mt * P : (mt + 1) * P, :], zt[:])

    nc.compile()
    return nc


_NC_CACHE = {}


def get_nc():
    if "nc" not in _NC_CACHE:
        _NC_CACHE["nc"] = build_nc()
    return _NC_CACHE["nc"]


def make_in_maps(Q, K, V, Wq, bq, Wk, bk, Wv, bv, Wo, bo):
    import ml_dtypes

    f = lambda x: np.asarray(x, dtype=np.float32)
    g = lambda x: np.ascontiguousarray(f(x).astype(ml_dtypes.bfloat16))
    Qb, Kb, Vb = g(Q), g(K), g(V)
    bo2 = f(bv).reshape(1, E) @ f(Wo) + f(bo).reshape(1, E)
    shared = {
        "wq": g(Wq), "wk": g(Wk), "wv": g(Wv), "wo": g(Wo),
        "bq": np.ascontiguousarray(f(bq).reshape(1, E)),
        "bk": np.ascontiguousarray(f(bk).reshape(1, E)),
        "bo": np.ascontiguousarray(bo2.astype(ml_dtypes.bfloat16)),
    }
    in_maps = []
    for c in range(NCORES):
        b, tq = divmod(c, 4)
        in_maps.append(
            {
                "q": np.ascontiguousarray(Qb[b, tq * TL : (tq + 1) * TL, :]),
                "k": Kb[b],
                "v": Vb[b],
                **shared,
            }
        )
    return in_maps


def assemble(results):
    out = np.empty((B, T, E), np.float32)
    for c in range(NCORES):
        b, tq = divmod(c, 4)
        out[b, tq * TL : (tq + 1) * TL, :] = results[c]["out"]
    return out


def kernel(Q, K, V, Wq, bq, Wk, bk, Wv, bv, Wo, bo):
    nc = get_nc()
    in_maps = make_in_maps(Q, K, V, Wq, bq, Wk, bk, Wv, bv, Wo, bo)
    res = run_bass_kernel_spmd(nc, in_maps, list(range(NCORES)))
    return assemble(res.results)
